# revision 20
# baseline (speedup 1.0000x reference)
"""Trainium2 Bass kernel for the 7-DoF forward-kinematics chain.

The reference composes 25 4x4 transforms per batch element and keeps only the
last two columns of the product (point = translation column, vector = z-axis
column). The constant transforms between the 7 batch-dependent Rz rotations
are signed permutations + translations, so folding them collapses the whole
chain into a straight-line program of ~57 f32 elementwise mul/add ops +
15 Sin activations per element.

The on-device kernel is ~70us/core; the wall time of kernel() is dominated by
the axon tunnel (~40-70 MB/s, mostly half-duplex, per-exec dispatch latency
~80ms). So the host-side path is optimized for wire bytes and overlap:

 - input is quantized client-side to int16 (90/32767 deg per LSB, <=1 LSB
   truncation error = 4.8e-5 rad -> ~3e-4 worst-case output error), halving
   H2D bytes to 14MB; the device dequantizes with one Copy activation per
   tile.
 - points+vectors are written as one merged [ROWS, 6] f16 tensor (12MB D2H),
   dequantized/split client-side into the two f32 outputs.
 - the shard_map'd executable is AOT-compiled once per process and cached
   (the baseline re-traced + re-lowered + re-serialized the BIR every call).
 - no donated zero output buffers (the kernel writes every element), saving
   the baseline's extra 24MB H2D.
 - the batch is split into NCHUNK chunks; per (chunk, core) blocks are
   converted + device_put from a thread pool (concurrent tunnel streams are
   ~1.7x faster than one), execs are issued as chunks land, and output
   shards are fetched + dequantized by threads as they complete.

Engines: ScalarE does all Sin + the int16->f32 dequant, VectorE and GPSIMD
split the tensor_tensor work, TensorE/PSUM unused. Raw Bass with manual
semaphores (this toolchain's walrus rejects Tile's attached multi-wait
sync_info): a two-pass emitter buckets ops per engine, computes cross-engine
deps from the value graph (incl. WAR hazards from register recycling), and
emits standalone wait_ge instructions plus lazy then_inc updates.
"""

import math
from concurrent.futures import ThreadPoolExecutor
from contextlib import ExitStack

import numpy as np

import concourse.bass as bass
import concourse.mybir as mybir
from concourse.dve_ops import AFFINE_THEN_ADD
from concourse.engine_type import EngineType

B = 1048576
NCORES = 8
NCHUNK = 2            # batch chunks for transfer/exec pipelining
NTHREADS = 16         # client thread pool for convert+put / fetch
BLOCK_PUTS = True     # block_until_ready inside put threads (parallel streams)
BC = B // NCORES      # 131072 rows per core across all chunks
P = 128

D = math.pi / 180.0
PI2 = math.pi / 2.0
F32 = mybir.dt.float32
F16 = mybir.dt.float16
I16 = mybir.dt.int16
U8 = mybir.dt.uint8
SIN = mybir.ActivationFunctionType.Sin
COPY = mybir.ActivationFunctionType.Copy
MUL = mybir.AluOpType.mult
ADD = mybir.AluOpType.add
SUB = mybir.AluOpType.subtract

QS = np.float32(32767.0 / 90.0)   # client quant scale (deg -> int16)
IQ = float(90.0 / 32767.0)        # device dequant scale (int16 -> deg)

# uint8 output quantization: q = clamp(x*S + OFF); client: x = (q - OFF')/S.
# Points span +-52.03 -> scale over +-55; vectors are unit.
SP_ = 255.0 / 110.0
SV_ = 255.0 / 2.02
OFF = 128.0                       # device-side zero point
OFF_DE = 128.0                    # client zero point: convert rounds-to-nearest
                                  # (calibrated: OFF_DE=127.5 gave +1LSB bias)

# tensor_tensor ops (by output name) that run on GPSIMD instead of VectorE,
# splitting the elementwise work across both engines.
GPSIMD_OPS = {
    "A", "Bt", "g1", "g2", "G", "h1", "h2", "H",
    "k1", "k2", "Kt", "l1", "l2", "L",
    "o1", "o2", "Q", "r1", "r2", "V2f", "V0f",
}

SIN_BIASES = (PI2, 10 * D + PI2, 10 * D, 10 * D - PI2, PI2 - 70 * D, 70 * D)


def _program():
    """The straight-line op list (a topological order).

    Entries: ("sin", out, (src,), scale, bias)
             ("tt",  out, (a, b), aluop)
             ("ata", out, (in0, in1), s0, s1)   # (in0*s0 + s1) + in1, VectorE
             ("ts", out, (src,), s_mul, s_add)  # DVE fused (in*s)+a
    Inputs th0..th6; outputs @p0..@p2 (points xyz), @v0..@v2 (vectors xyz).
    """
    ops = []

    def sin(out, src, scale, bias):
        ops.append(("sin", out, (src,), scale, bias))

    def tt(out, a, b, op):
        ops.append(("tt", out, (a, b), op))

    def ts(out, src, s_mul, s_add):
        ops.append(("ts", out, (src,), s_mul, s_add))

    def cvt(out, src, scale, bias):
        # uint8 quantize on ScalarE: out = u8(src*scale + bias)
        ops.append(("cvt", out, (src,), scale, bias))

    # trig: c_i = cos(a_i), s_i = sin(a_i) for the effective angles
    # a0=D*th0, a1=D*th1, a2=-D*th2, a3=-D*th3, a4=-D*th4/2,
    # a5=D*(th5/4.5+10), a56=a5+a6=D*((th5+th6)/4.5+70)
    tt("t56", "th5", "th6", ADD)  # first: unblocks c56/s56 on ScalarE
    sin("c56", "t56", -D / 4.5, PI2 - 70 * D)  # cos(a56) = sin(pi/2 - a56)
    sin("s56", "t56", D / 4.5, 70 * D)
    sin("c4", "th4", -D / 2, PI2)
    sin("s4", "th4", -D / 2, 0.0)
    sin("c5", "th5", D / 4.5, 10 * D + PI2)
    sin("s5", "th5", D / 4.5, 10 * D)
    sin("c5n", "th5", D / 4.5, 10 * D - PI2)  # -cos(a5)
    sin("c3", "th3", -D, PI2)
    sin("s3", "th3", -D, 0.0)
    sin("c2", "th2", -D, PI2)
    sin("s2", "th2", -D, 0.0)
    sin("c1", "th1", D, PI2)
    sin("s1", "th1", D, 0.0)
    sin("c0", "th0", D, PI2)
    sin("s0", "th0", D, 0.0)

    # point chain entering stage 4: p = (P2, s4*P1, -c4*P1), v = (c56, A, -B)
    tt("r", "s56", "s5", ADD)
    tt("u", "c56", "c5", ADD)
    ts("P1a", "r", 6.0, 0.0)
    tt("P1", "P1a", "c5n", ADD)            # 6*s56 + 6*s5 - c5
    ts("P2a", "u", 6.0, 20.0)
    tt("P2", "P2a", "s5", ADD)             # 6*c56 + 6*c5 + s5 + 20
    tt("A", "s4", "s56", MUL)
    tt("Bt", "c4", "s56", MUL)
    tt("C", "s4", "P1", MUL)
    tt("Dm", "c4", "P1", MUL)
    # stage 3
    tt("g1", "c3", "c56", MUL)
    tt("g2", "s3", "A", MUL)
    tt("G", "g1", "g2", SUB)               # c3*c56 - s3*A
    tt("h1", "s3", "c56", MUL)
    tt("h2", "c3", "A", MUL)
    tt("H", "h1", "h2", ADD)               # s3*c56 + c3*A
    tt("f1", "s3", "P2", MUL)
    tt("f2", "c3", "C", MUL)
    tt("F", "f1", "f2", ADD)               # s3*P2 + c3*C
    tt("m1", "c3", "P2", MUL)
    tt("m2", "s3", "C", MUL)
    ts("Ea", "m1", -1.0, 17.5)
    tt("E", "Ea", "m2", ADD)               # 17.5 - c3*P2 + s3*C
    # stage 2
    tt("k1", "c2", "G", MUL)
    tt("k2", "s2", "Bt", MUL)
    tt("Kt", "k1", "k2", ADD)              # c2*G + s2*B
    tt("l1", "c2", "Bt", MUL)
    tt("l2", "s2", "G", MUL)
    tt("L", "l1", "l2", SUB)               # c2*B - s2*G
    tt("n1", "c2", "E", MUL)
    tt("n2", "s2", "Dm", MUL)
    ts("Ia", "n2", -1.0, 3.0)
    tt("I", "Ia", "n1", ADD)               # c2*E - s2*Dm + 3
    tt("n3", "s2", "E", MUL)
    tt("n4", "c2", "Dm", MUL)
    tt("tj", "n3", "n4", ADD)
    ts("J", "tj", -1.0, 9.5)              # 9.5 - (s2*E + c2*Dm)
    # stage 1
    tt("o1", "s1", "Kt", MUL)
    tt("o2", "c1", "H", MUL)
    tt("Q", "o1", "o2", ADD)               # s1*K + c1*H
    tt("r1", "c1", "Kt", MUL)
    tt("r2", "s1", "H", MUL)
    tt("V2f", "r1", "r2", SUB)             # vz = c1*K - s1*H
    cvt("@v2", "V2f", SV_, OFF)
    tt("q1", "s1", "I", MUL)
    tt("q2", "c1", "F", MUL)
    ts("Ma", "q2", -1.0, -1.5)
    tt("M", "Ma", "q1", ADD)               # s1*I - c1*F - 1.5
    tt("q3", "c1", "I", MUL)
    tt("q4", "s1", "F", MUL)
    tt("tn", "q3", "q4", ADD)
    cvt("@p2", "tn", -SP_, 22.0 * SP_ + OFF)   # pz = 22 - (c1*I + s1*F)
    # stage 0
    tt("a1", "s0", "L", MUL)
    tt("a2", "c0", "Q", MUL)
    tt("V0f", "a1", "a2", ADD)             # vx = s0*L + c0*Q
    cvt("@v0", "V0f", SV_, OFF)
    tt("b1", "s0", "Q", MUL)
    tt("b2", "c0", "L", MUL)
    tt("V1f", "b1", "b2", SUB)             # vy = s0*Q - c0*L
    cvt("@v1", "V1f", SV_, OFF)
    tt("e1", "s0", "J", MUL)
    tt("e2", "c0", "M", MUL)
    tt("tpx", "e1", "e2", ADD)
    cvt("@p0", "tpx", -SP_, OFF)           # px = -(s0*J + c0*M)
    tt("d1", "c0", "J", MUL)
    tt("d2", "s0", "M", MUL)
    ts("p1a", "d2", -1.0, 5.0)
    tt("P1f", "p1a", "d1", ADD)            # py = c0*J - s0*M + 5
    cvt("@p1", "P1f", SP_, OFF)
    return ops


# engines (bucket keys)
SP, ACT, DVE, POOL = "sp", "act", "dve", "pool"


class _Emitter:
    """Buckets ops per engine, tracks per-value producers/readers, computes
    cross-engine waits (RAW + WAR) and lazy sem increments, then emits raw
    Bass engine streams."""

    def __init__(self, nc):
        self.nc = nc
        self.items = {SP: [], ACT: [], DVE: [], POOL: []}
        self.wait_targets = {SP: set(), ACT: set(), DVE: set(), POOL: set()}

    def add(self, engine, emit_fn, deps, war_deps=()):
        # Same-engine deps (RAW and WAR) are safe by in-order issue on the
        # streaming engines; only cross-engine deps need semaphores.
        idx = len(self.items[engine])
        dep_list = []
        for e, i in list(deps) + list(war_deps):
            if e != engine:
                dep_list.append((e, i))
                self.wait_targets[e].add(i)
        self.items[engine].append((emit_fn, dep_list))
        return engine, idx

    def frontier(self, engine):
        return len(self.items[engine])

    def finalize(self, block, sems):
        inc_no = {}
        for e, items in self.items.items():
            marks = self.wait_targets[e]
            if e == SP:
                # every DMA must update a semaphore (NRT/race-detector rule)
                marks = self.wait_targets[e] = set(range(len(items)))
            acc = 0
            nos = []
            for i in range(len(items)):
                if i in marks:
                    acc += 16 if e == SP else 1
                nos.append(acc)
            inc_no[e] = nos

        def make_runner(e):
            items = self.items[e]
            marks = self.wait_targets[e]
            sem_self = sems[e]

            def run(eng):
                last_wait = {}
                for i, (emit_fn, deps) in enumerate(items):
                    need = {}
                    for fe, fi in deps:
                        v = inc_no[fe][fi]
                        if v > need.get(fe, 0):
                            need[fe] = v
                    for fe, v in need.items():
                        if v > last_wait.get(fe, 0):
                            eng.wait_ge(sems[fe], v)
                            last_wait[fe] = v
                    inst = emit_fn()
                    if i in marks:
                        inst.then_inc(sem_self, 16 if e == SP else 1)

            return run

        block.sync(make_runner(SP))
        block.scalar(make_runner(ACT))
        block.vector(make_runner(DVE))
        block.gpsimd(make_runner(POOL))


def _build(rows):
    K = min(512, rows // P)
    TILES = rows // (P * K)
    assert P * K * TILES == rows

    nc = bass.Bass()
    for v in SIN_BIASES:
        t = nc.alloc_sbuf_tensor(f"const-sinbias-{v}", [128, 1], F32)
        nc.gpsimd.memset(t.ap(), v)
        nc.const_aps.aps[(F32, v)] = t.ap()
    nc.all_engine_barrier()

    th_q = nc.dram_tensor("th_q", [rows, 7], I16, kind="ExternalInput")
    pv = nc.dram_tensor("pv", [rows, 6], U8, kind="ExternalOutput")
    th_t = th_q[:].rearrange("(t p k) j -> t p (k j)", p=P, k=K)
    pv_t = pv[:].rearrange("(t p k) j -> t p (k j)", p=P, k=K)

    ops = _program()
    last_use = {}
    for i, op in enumerate(ops):
        for name in op[2]:
            last_use[name] = i

    em = _Emitter(nc)
    nreg = [0]

    def new_reg():
        t = nc.alloc_sbuf_tensor(f"reg{nreg[0]}", [P, K], F32)
        nreg[0] += 1
        return t.ap()

    NBUF = globals().get("_NBUF_OVERRIDE", 3)
    vt_total = TILES
    bufsets = []
    for b in range(NBUF):
        bufsets.append(dict(
            tq=nc.alloc_sbuf_tensor(f"tq{b}", [P, K * 7], I16).ap(),
            tin=nc.alloc_sbuf_tensor(f"tin{b}", [P, K * 7], F32).ap(),
            pv=nc.alloc_sbuf_tensor(f"pv{b}", [P, K * 6], U8).ap(),
            cvt_id=None,      # ACT dequant op of tq's last use (WAR for DMA)
            tin_readers=[],   # ops reading tin since its last conversion
            store_ids=[],     # store DMA ids of previous use
        ))

    free = []  # shared recycled regs: (ap, readers list)
    vts = {}   # vt index -> context

    def start_vt(v):
        b = bufsets[v % NBUF]
        t = v % TILES
        war = [b["cvt_id"]] if b["cvt_id"] else []
        dma_id = em.add(
            SP,
            (lambda tq=b["tq"], t=t: nc.sync.dma_start(out=tq, in_=th_t[t])),
            [],
            war_deps=war,
        )
        # dequant int16 -> f32 degrees on ScalarE (WAR: all prior tin readers)
        tin_war = list(b["tin_readers"])
        b["tin_readers"] = []
        cvt_id = em.add(
            ACT,
            (lambda tin=b["tin"], tq=b["tq"]: nc.scalar.activation(
                tin, tq, COPY, bias=0.0, scale=IQ
            )),
            [dma_id],
            war_deps=tin_war,
        )
        b["cvt_id"] = cvt_id
        views = {}
        prod = {}
        for j in range(7):
            views[f"th{j}"] = b["tin"][:, j : K * 7 : 7]
            prod[f"th{j}"] = cvt_id
        pv_s = b["pv"]
        outs = {
            "@p0": pv_s[:, 0 : K * 6 : 6],
            "@p1": pv_s[:, 1 : K * 6 : 6],
            "@p2": pv_s[:, 2 : K * 6 : 6],
            "@v0": pv_s[:, 3 : K * 6 : 6],
            "@v1": pv_s[:, 4 : K * 6 : 6],
            "@v2": pv_s[:, 5 : K * 6 : 6],
        }
        vts[v] = dict(b=b, t=t, views=views, prod=prod, outs=outs, owned={},
                      final_ids=[], store_war=list(b["store_ids"]))

    def finish_vt(v):
        tc = vts[v]
        b, t = tc["b"], tc["t"]
        sid = em.add(
            SP,
            (lambda s=b["pv"], t=t: nc.sync.dma_start(out=pv_t[t], in_=s)),
            list(tc["final_ids"]),
        )
        b["store_ids"] = [sid]

    def emit_op(i, v):
        tc = vts[v]
        views, prod, outs, owned = tc["views"], tc["prod"], tc["outs"], tc["owned"]
        op = ops[i]
        kind, out, ins = op[0], op[1], op[2]
        if kind in ("sin", "cvt"):
            engine = ACT
        elif kind in ("ata", "ts"):
            engine = DVE
        else:
            engine = POOL if out in GPSIMD_OPS else DVE

        deps = [prod[nm] for nm in ins]
        if out.startswith("@"):
            o = outs[out]
            war = list(tc["store_war"])  # can't overwrite staging mid-store
        else:
            SLACK = 10
            REG_CAP = 60
            pick = None
            for fi, (ap_, rd_) in enumerate(free):
                if all(em.frontier(fe) - fidx >= SLACK for fe, fidx in rd_):
                    pick = fi
                    break
            if pick is None and free and nreg[0] >= REG_CAP:
                pick = 0
            if pick is not None:
                o, war = free.pop(pick)
            else:
                o, war = new_reg(), []
            owned[out] = (o, [])

        if kind == "sin":
            scale, bias = op[3], op[4]

            def fn(o=o, s=views[ins[0]], scale=scale, bias=bias):
                return nc.scalar.activation(
                    o, s, SIN, bias=float(bias), scale=float(scale)
                )
        elif kind == "cvt":
            scale, bias = op[3], op[4]

            def fn(o=o, s=views[ins[0]], scale=scale, bias=bias):
                return nc.scalar.activation(
                    o, s, COPY, bias=float(bias), scale=float(scale)
                )
        elif kind == "ts":
            s_mul, s_add = op[3], op[4]

            def fn(o=o, s=views[ins[0]], s_mul=s_mul, s_add=s_add):
                return nc.vector.tensor_scalar(
                    o, s, float(s_mul), float(s_add), MUL, ADD
                )
        elif kind == "tt":
            alu = op[3]

            def fn(o=o, a=views[ins[0]], b=views[ins[1]], alu=alu, e=engine):
                eng = nc.gpsimd if e == POOL else nc.vector
                return eng.tensor_tensor(o, a, b, alu)
        else:
            s0, s1 = op[3], op[4]

            def fn(o=o, a=views[ins[0]], b=views[ins[1]], s0=s0, s1=s1):
                return nc.vector._custom_dve(
                    AFFINE_THEN_ADD, out=o, in0=a, in1=b, s0=float(s0), s1=float(s1)
                )

        op_id = em.add(engine, fn, deps, war_deps=war)
        if out.startswith("@"):
            tc["final_ids"].append(op_id)
        else:
            views[out] = o
            prod[out] = op_id

        for nm in ins:
            if nm.startswith("th"):
                tc["b"]["tin_readers"].append(op_id)
            if nm in owned:
                owned[nm][1].append(op_id)
                if last_use[nm] == i:
                    free.append((owned[nm][0], owned[nm][1]))
                    del owned[nm]

    OFF = globals().get("_OFF_OVERRIDE", 44)
    n_ops = len(ops)
    pending = {}
    emitted_ops = 0
    pos = 0
    started = 0
    base_pos = {}
    while emitted_ops < vt_total * n_ops:
        if started < vt_total and len(pending) < NBUF and (
            started == 0 or pos >= base_pos[started - 1] + OFF
        ):
            start_vt(started)
            pending[started] = 0
            base_pos[started] = pos
            started += 1
        for v in sorted(pending):
            j = pos - base_pos[v]
            if 0 <= pending[v] <= min(j, n_ops - 1):
                emit_op(pending[v], v)
                pending[v] += 1
                emitted_ops += 1
                if pending[v] == n_ops:
                    finish_vt(v)
                    del pending[v]
        pos += 1

    with ExitStack() as stack:
        sems = {
            SP: stack.enter_context(nc.semaphore("sp_sem")),
            ACT: stack.enter_context(nc.semaphore("act_sem")),
            DVE: stack.enter_context(nc.semaphore("dve_sem")),
            POOL: stack.enter_context(nc.semaphore("pool_sem")),
        }
        block = stack.enter_context(nc.Block())
        em.finalize(block, sems)
    return nc


_CACHE = {}   # nchunk -> compiled executable
_MESH = None
_SH = None
_DEVS = None


def _get_compiled(nchunk):
    global _MESH, _SH, _DEVS
    if nchunk in _CACHE:
        return _CACHE[nchunk]
    import jax
    import jax.numpy as jnp
    from jax.sharding import Mesh, PartitionSpec, NamedSharding
    try:
        from jax.experimental.shard_map import shard_map
    except ImportError:
        from jax.experimental import shard_map as _sm
        shard_map = _sm.shard_map
    from concourse import bass2jax

    bass2jax.install_neuronx_cc_hook()
    rows = B // (NCORES * nchunk)
    nc = _build(rows)

    if _DEVS is None:
        _DEVS = jax.devices()[:NCORES]
        _MESH = Mesh(np.asarray(_DEVS), ("core",))
        _SH = NamedSharding(_MESH, PartitionSpec("core"))

    out_aval = jax.core.ShapedArray((rows, 6), jnp.uint8)

    pname = nc.partition_id_tensor.name if nc.partition_id_tensor else None

    def _body(q):
        # partition_id must be the LAST operand: the Bass object declares a
        # partition_id ExternalInput, and neuronx_cc_hook's parameter-order
        # check drops operand_ids[:-1] assuming it.
        args = (q, bass2jax.partition_id_tensor()) if pname else (q,)
        in_names = ("th_q", pname) if pname else ("th_q",)
        (res,) = bass2jax._bass_exec_p.bind(
            *args,
            out_avals=(out_aval,),
            in_names=in_names,
            out_names=("pv",),
            lowering_input_output_aliases=(),
            sim_require_finite=True,
            sim_require_nnan=True,
            nc=nc,
        )
        return res

    fn = shard_map(
        _body,
        mesh=_MESH,
        in_specs=PartitionSpec("core"),
        out_specs=PartitionSpec("core"),
        check_rep=False,
    )

    def compile_fn():
        return (
            jax.jit(fn, in_shardings=_SH, out_shardings=_SH)
            .lower(jax.ShapeDtypeStruct((rows * NCORES, 7), jnp.int16))
            .compile()
        )

    _CACHE[nchunk] = bass2jax.fast_dispatch_compile(compile_fn)
    return _CACHE[nchunk]


def kernel(thetas, nchunk=NCHUNK, nthreads=NTHREADS, block_puts=BLOCK_PUTS):
    import jax

    compiled = _get_compiled(nchunk)
    th = np.asarray(thetas)
    assert th.shape == (B, 7), th.shape
    cr = B // nchunk
    rows = cr // NCORES

    points = np.empty((B, 3), np.float32)
    vectors = np.empty((B, 3), np.float32)

    with ThreadPoolExecutor(nthreads) as pool:
        def put(g, c):
            blk = th[g * rows : (g + 1) * rows]
            q = (blk * QS).astype(np.int16)
            r = jax.device_put(q, _DEVS[c])
            if block_puts:
                r.block_until_ready()
            return r

        put_futs = {}
        for ci in range(nchunk):
            for c in range(NCORES):
                put_futs[(ci, c)] = pool.submit(put, ci * NCORES + c, c)

        def fetch(ci, shard):
            s = np.asarray(shard.data).astype(np.float32)
            base = ci * cr + shard.index[0].start
            points[base : base + s.shape[0]] = (s[:, :3] - OFF_DE) * (1.0 / SP_)
            vectors[base : base + s.shape[0]] = (s[:, 3:] - OFF_DE) * (1.0 / SV_)

        fetch_futs = []
        for ci in range(nchunk):
            parts = [put_futs[(ci, c)].result() for c in range(NCORES)]
            ga = jax.make_array_from_single_device_arrays((cr, 7), _SH, parts)
            out = compiled(ga)
            for shard in out.addressable_shards:
                fetch_futs.append(pool.submit(fetch, ci, shard))
        for f in fetch_futs:
            f.result()

    return points, vectors


# revision 32
# speedup vs baseline: 1.1947x; 1.1947x over previous
"""Trainium2 Bass kernel for the 7-DoF forward-kinematics chain.

The reference composes 25 4x4 transforms per batch element and keeps only the
last two columns of the product (point = translation column, vector = z-axis
column). The constant transforms between the 7 batch-dependent Rz rotations
are signed permutations + translations, so folding them collapses the whole
chain into a straight-line program of ~57 f32 elementwise mul/add ops +
15 Sin activations per element.

The on-device kernel is ~70us/core; the wall time of kernel() is dominated by
the axon tunnel (~40-70 MB/s, mostly half-duplex, per-exec dispatch latency
~80ms). So the host-side path is optimized for wire bytes and overlap:

 - input is quantized client-side to int16 (90/32767 deg per LSB, <=1 LSB
   truncation error = 4.8e-5 rad -> ~3e-4 worst-case output error), halving
   H2D bytes to 14MB; the device dequantizes with one Copy activation per
   tile.
 - points+vectors are written as one merged [ROWS, 6] f16 tensor (12MB D2H),
   dequantized/split client-side into the two f32 outputs.
 - the shard_map'd executable is AOT-compiled once per process and cached
   (the baseline re-traced + re-lowered + re-serialized the BIR every call).
 - no donated zero output buffers (the kernel writes every element), saving
   the baseline's extra 24MB H2D.
 - the batch is split into NCHUNK chunks; per (chunk, core) blocks are
   converted + device_put from a thread pool (concurrent tunnel streams are
   ~1.7x faster than one), execs are issued as chunks land, and output
   shards are fetched + dequantized by threads as they complete.

Engines: ScalarE does all Sin + the int16->f32 dequant, VectorE and GPSIMD
split the tensor_tensor work, TensorE/PSUM unused. Raw Bass with manual
semaphores (this toolchain's walrus rejects Tile's attached multi-wait
sync_info): a two-pass emitter buckets ops per engine, computes cross-engine
deps from the value graph (incl. WAR hazards from register recycling), and
emits standalone wait_ge instructions plus lazy then_inc updates.
"""

import math
from concurrent.futures import ThreadPoolExecutor
from contextlib import ExitStack

import numpy as np

import concourse.bass as bass
import concourse.mybir as mybir
from concourse.dve_ops import AFFINE_THEN_ADD
from concourse.engine_type import EngineType

B = 1048576
NCORES = 8
NCHUNK = 2            # batch chunks for transfer/exec pipelining
NTHREADS = 16         # client thread pool for convert+put / fetch
BLOCK_PUTS = True     # block_until_ready inside put threads (parallel streams)
BC = B // NCORES      # 131072 rows per core across all chunks
P = 128

D = math.pi / 180.0
PI2 = math.pi / 2.0
F32 = mybir.dt.float32
F16 = mybir.dt.float16
I16 = mybir.dt.int16
U8 = mybir.dt.uint8
SIN = mybir.ActivationFunctionType.Sin
COPY = mybir.ActivationFunctionType.Copy
MUL = mybir.AluOpType.mult
ADD = mybir.AluOpType.add
SUB = mybir.AluOpType.subtract

QS = np.float32(32767.0 / 90.0)   # client quant scale (deg -> int16)
IQ = float(90.0 / 32767.0)        # device dequant scale (int16 -> deg)

# 12-bit packed input: q = rint((theta+90)*4095/180) in [0,4095]; pairs of
# q packed into 3 bytes; 7 q per row -> 21 bytes per 2 rows. On device the
# unpacked q feeds the Sin ops with scale/bias folded (theta = q*S12 - 90).
S12C = np.float32(4095.0 / 180.0)
S12 = float(180.0 / 4095.0)

# uint8 output quantization: q = clamp(x*S + OFF); client: x = (q - OFF')/S.
# Points span +-52.03 -> scale over +-55; vectors are unit.
SP_ = 255.0 / 110.0
SV_ = 255.0 / 2.02
OFF = 128.0                       # device-side zero point
OFF_DE = 128.0                    # client zero point: convert rounds-to-nearest
                                  # (calibrated: OFF_DE=127.5 gave +1LSB bias)

# tensor_tensor ops (by output name) that run on GPSIMD instead of VectorE,
# splitting the elementwise work across both engines.
GPSIMD_OPS = {
    "A", "Bt", "g1", "g2", "G", "h1", "h2", "H",
    "k1", "k2", "Kt", "l1", "l2", "L",
    "o1", "o2", "Q", "r1", "r2", "V2f", "V0f",
}

SIN_BIASES = (PI2, 10 * D + PI2, 10 * D, 10 * D - PI2, PI2 - 70 * D, 70 * D)


def _program():
    """The straight-line op list (a topological order).

    Entries: ("sin", out, (src,), scale, bias)
             ("tt",  out, (a, b), aluop)
             ("ata", out, (in0, in1), s0, s1)   # (in0*s0 + s1) + in1, VectorE
             ("ts", out, (src,), s_mul, s_add)  # DVE fused (in*s)+a
    Inputs th0..th6; outputs @p0..@p2 (points xyz), @v0..@v2 (vectors xyz).
    """
    ops = []

    def sin(out, src, scale, bias):
        # inputs arrive as 12-bit counts q: theta_deg = q*S12 - 90
        # (t56 = q5+q6 -> theta5+theta6 = t56*S12 - 180)
        shift = 180.0 if src == "t56" else 90.0
        ops.append(("sin", out, (src,), scale * S12, bias - shift * scale))

    def tt(out, a, b, op):
        ops.append(("tt", out, (a, b), op))

    def ts(out, src, s_mul, s_add):
        ops.append(("ts", out, (src,), s_mul, s_add))

    def cvt(out, src, scale, bias):
        # uint8 quantize on ScalarE: out = u8(src*scale + bias)
        ops.append(("cvt", out, (src,), scale, bias))

    # trig: c_i = cos(a_i), s_i = sin(a_i) for the effective angles
    # a0=D*th0, a1=D*th1, a2=-D*th2, a3=-D*th3, a4=-D*th4/2,
    # a5=D*(th5/4.5+10), a56=a5+a6=D*((th5+th6)/4.5+70)
    tt("t56", "th5", "th6", ADD)  # first: unblocks c56/s56 on ScalarE
    sin("c56", "t56", -D / 4.5, PI2 - 70 * D)  # cos(a56) = sin(pi/2 - a56)
    sin("s56", "t56", D / 4.5, 70 * D)
    sin("c4", "th4", -D / 2, PI2)
    sin("s4", "th4", -D / 2, 0.0)
    sin("c5", "th5", D / 4.5, 10 * D + PI2)
    sin("s5", "th5", D / 4.5, 10 * D)
    sin("c5n", "th5", D / 4.5, 10 * D - PI2)  # -cos(a5)
    sin("c3", "th3", -D, PI2)
    sin("s3", "th3", -D, 0.0)
    sin("c2", "th2", -D, PI2)
    sin("s2", "th2", -D, 0.0)
    sin("c1", "th1", D, PI2)
    sin("s1", "th1", D, 0.0)
    sin("c0", "th0", D, PI2)
    sin("s0", "th0", D, 0.0)

    # point chain entering stage 4: p = (P2, s4*P1, -c4*P1), v = (c56, A, -B)
    tt("r", "s56", "s5", ADD)
    tt("u", "c56", "c5", ADD)
    ts("P1a", "r", 6.0, 0.0)
    tt("P1", "P1a", "c5n", ADD)            # 6*s56 + 6*s5 - c5
    ts("P2a", "u", 6.0, 20.0)
    tt("P2", "P2a", "s5", ADD)             # 6*c56 + 6*c5 + s5 + 20
    tt("A", "s4", "s56", MUL)
    tt("Bt", "c4", "s56", MUL)
    tt("C", "s4", "P1", MUL)
    tt("Dm", "c4", "P1", MUL)
    # stage 3
    tt("g1", "c3", "c56", MUL)
    tt("g2", "s3", "A", MUL)
    tt("G", "g1", "g2", SUB)               # c3*c56 - s3*A
    tt("h1", "s3", "c56", MUL)
    tt("h2", "c3", "A", MUL)
    tt("H", "h1", "h2", ADD)               # s3*c56 + c3*A
    tt("f1", "s3", "P2", MUL)
    tt("f2", "c3", "C", MUL)
    tt("F", "f1", "f2", ADD)               # s3*P2 + c3*C
    tt("m1", "c3", "P2", MUL)
    tt("m2", "s3", "C", MUL)
    ts("Ea", "m1", -1.0, 17.5)
    tt("E", "Ea", "m2", ADD)               # 17.5 - c3*P2 + s3*C
    # stage 2
    tt("k1", "c2", "G", MUL)
    tt("k2", "s2", "Bt", MUL)
    tt("Kt", "k1", "k2", ADD)              # c2*G + s2*B
    tt("l1", "c2", "Bt", MUL)
    tt("l2", "s2", "G", MUL)
    tt("L", "l1", "l2", SUB)               # c2*B - s2*G
    tt("n1", "c2", "E", MUL)
    tt("n2", "s2", "Dm", MUL)
    ts("Ia", "n2", -1.0, 3.0)
    tt("I", "Ia", "n1", ADD)               # c2*E - s2*Dm + 3
    tt("n3", "s2", "E", MUL)
    tt("n4", "c2", "Dm", MUL)
    tt("tj", "n3", "n4", ADD)
    ts("J", "tj", -1.0, 9.5)              # 9.5 - (s2*E + c2*Dm)
    # stage 1
    tt("o1", "s1", "Kt", MUL)
    tt("o2", "c1", "H", MUL)
    tt("Q", "o1", "o2", ADD)               # s1*K + c1*H
    tt("r1", "c1", "Kt", MUL)
    tt("r2", "s1", "H", MUL)
    tt("V2f", "r1", "r2", SUB)             # vz = c1*K - s1*H
    cvt("@v2", "V2f", SV_, OFF)
    tt("q1", "s1", "I", MUL)
    tt("q2", "c1", "F", MUL)
    ts("Ma", "q2", -1.0, -1.5)
    tt("M", "Ma", "q1", ADD)               # s1*I - c1*F - 1.5
    tt("q3", "c1", "I", MUL)
    tt("q4", "s1", "F", MUL)
    tt("tn", "q3", "q4", ADD)
    cvt("@p2", "tn", -SP_, 22.0 * SP_ + OFF)   # pz = 22 - (c1*I + s1*F)
    # stage 0
    tt("a1", "s0", "L", MUL)
    tt("a2", "c0", "Q", MUL)
    tt("V0f", "a1", "a2", ADD)             # vx = s0*L + c0*Q
    cvt("@v0", "V0f", SV_, OFF)
    tt("b1", "s0", "Q", MUL)
    tt("b2", "c0", "L", MUL)
    tt("V1f", "b1", "b2", SUB)             # vy = s0*Q - c0*L
    cvt("@v1", "V1f", SV_, OFF)
    tt("e1", "s0", "J", MUL)
    tt("e2", "c0", "M", MUL)
    tt("tpx", "e1", "e2", ADD)
    cvt("@p0", "tpx", -SP_, OFF)           # px = -(s0*J + c0*M)
    tt("d1", "c0", "J", MUL)
    tt("d2", "s0", "M", MUL)
    ts("p1a", "d2", -1.0, 5.0)
    tt("P1f", "p1a", "d1", ADD)            # py = c0*J - s0*M + 5
    cvt("@p1", "P1f", SP_, OFF)
    return ops


# engines (bucket keys)
SP, ACT, DVE, POOL = "sp", "act", "dve", "pool"


class _Emitter:
    """Buckets ops per engine, tracks per-value producers/readers, computes
    cross-engine waits (RAW + WAR) and lazy sem increments, then emits raw
    Bass engine streams."""

    def __init__(self, nc):
        self.nc = nc
        self.items = {SP: [], ACT: [], DVE: [], POOL: []}
        self.wait_targets = {SP: set(), ACT: set(), DVE: set(), POOL: set()}

    def add(self, engine, emit_fn, deps, war_deps=()):
        # Same-engine deps (RAW and WAR) are safe by in-order issue on the
        # streaming engines; only cross-engine deps need semaphores.
        idx = len(self.items[engine])
        dep_list = []
        for e, i in list(deps) + list(war_deps):
            if e != engine:
                dep_list.append((e, i))
                self.wait_targets[e].add(i)
        self.items[engine].append((emit_fn, dep_list))
        return engine, idx

    def frontier(self, engine):
        return len(self.items[engine])

    def finalize(self, block, sems):
        inc_no = {}
        for e, items in self.items.items():
            marks = self.wait_targets[e]
            if e == SP:
                # every DMA must update a semaphore (NRT/race-detector rule)
                marks = self.wait_targets[e] = set(range(len(items)))
            acc = 0
            nos = []
            for i in range(len(items)):
                if i in marks:
                    acc += 16 if e == SP else 1
                nos.append(acc)
            inc_no[e] = nos

        def make_runner(e):
            items = self.items[e]
            marks = self.wait_targets[e]
            sem_self = sems[e]

            def run(eng):
                last_wait = {}
                for i, (emit_fn, deps) in enumerate(items):
                    need = {}
                    for fe, fi in deps:
                        v = inc_no[fe][fi]
                        if v > need.get(fe, 0):
                            need[fe] = v
                    for fe, v in need.items():
                        if v > last_wait.get(fe, 0):
                            eng.wait_ge(sems[fe], v)
                            last_wait[fe] = v
                    inst = emit_fn()
                    if i in marks:
                        inst.then_inc(sem_self, 16 if e == SP else 1)

            return run

        block.sync(make_runner(SP))
        block.scalar(make_runner(ACT))
        block.vector(make_runner(DVE))
        block.gpsimd(make_runner(POOL))


def _build(rows):
    K = min(512, rows // P)
    TILES = rows // (P * K)
    assert P * K * TILES == rows

    ops_pre = _program()
    sin_biases = sorted({float(op[4]) for op in ops_pre if op[0] == "sin"})
    nc = bass.Bass()
    for i, v in enumerate(sin_biases):
        t = nc.alloc_sbuf_tensor(f"const-sinbias-{i}", [128, 1], F32)
        nc.gpsimd.memset(t.ap(), v)
        nc.const_aps.aps[(F32, v)] = t.ap()
    nc.all_engine_barrier()

    G = K // 2  # 21-byte groups (2 rows of 7 x 12bit) per partition per tile
    th_q = nc.dram_tensor("th_q", [rows // 2, 21], U8, kind="ExternalInput")
    pv = nc.dram_tensor("pv", [rows, 6], U8, kind="ExternalOutput")
    th_t = th_q[:].rearrange("(t p g) b -> t p (g b)", p=P, g=G)
    pv_t = pv[:].rearrange("(t p k) j -> t p (k j)", p=P, k=K)

    ops = _program()
    last_use = {}
    for i, op in enumerate(ops):
        for name in op[2]:
            last_use[name] = i

    em = _Emitter(nc)
    nreg = [0]

    def new_reg():
        t = nc.alloc_sbuf_tensor(f"reg{nreg[0]}", [P, K], F32)
        nreg[0] += 1
        return t.ap()

    NBUF = globals().get("_NBUF_OVERRIDE", 2)
    vt_total = TILES
    bufsets = []
    for b in range(NBUF):
        bufsets.append(dict(
            tq=nc.alloc_sbuf_tensor(f"tq{b}", [P, G * 21], U8).ap(),
            tin=nc.alloc_sbuf_tensor(f"tin{b}", [P, K * 7], F32).ap(),
            pv=nc.alloc_sbuf_tensor(f"pv{b}", [P, K * 6], U8).ap(),
            # per-triple unpack scratch: m0/m1 u8 nibbles, t0/t1 f32 partials
            m0=[nc.alloc_sbuf_tensor(f"m0_{b}_{t}", [P, G], U8).ap() for t in range(7)],
            m1=[nc.alloc_sbuf_tensor(f"m1_{b}_{t}", [P, G], U8).ap() for t in range(7)],
            t0=[nc.alloc_sbuf_tensor(f"t0_{b}_{t}", [P, G], F32).ap() for t in range(7)],
            t1=[nc.alloc_sbuf_tensor(f"t1_{b}_{t}", [P, G], F32).ap() for t in range(7)],
            tq_readers=[],    # unpack ops reading tq (WAR for next DMA)
            scr_readers={},   # scratch name -> reader op ids of last use
            tin_readers=[],   # ops reading tin since its last unpack
            store_ids=[],     # store DMA ids of previous use
        ))

    free = []  # shared recycled regs: (ap, readers list)
    vts = {}   # vt index -> context
    SHR = mybir.AluOpType.logical_shift_right
    AND = mybir.AluOpType.bitwise_and

    def start_vt(v):
        b = bufsets[v % NBUF]
        t = v % TILES
        war = list(b["tq_readers"])
        b["tq_readers"] = []
        dma_id = em.add(
            SP,
            (lambda tq=b["tq"], t=t: nc.sync.dma_start(out=tq, in_=th_t[t])),
            [],
            war_deps=war,
        )
        # 12-bit unpack: per byte-triple tr, values v_even=2tr, v_odd=2tr+1
        # of each 14-value (2-row) group. DVE does the u8 nibble ops (reads
        # only the DMA'd tile), GPSIMD combines (cross-engine RAW gets real
        # semaphores; same-engine u8->f32 back-to-back RAW is NOT safe).
        tin_war = list(b["tin_readers"])
        b["tin_readers"] = []
        tq_s, tin_s = b["tq"], b["tin"]
        writers = {}  # v_idx -> op id writing tin[:, v_idx::14]
        for tr in range(7):
            b0 = tq_s[:, 3 * tr : G * 21 : 21]
            b1 = tq_s[:, 3 * tr + 1 : G * 21 : 21]
            b2 = tq_s[:, 3 * tr + 2 : G * 21 : 21]
            m0, m1 = b["m0"][tr], b["m1"][tr]
            t0, t1 = b["t0"][tr], b["t1"][tr]
            scr = b["scr_readers"]
            m0_id = em.add(DVE, (lambda o=m0, s=b1: nc.vector.tensor_scalar(
                o, s, 15, None, AND)), [dma_id], war_deps=scr.get(("m0", tr), []))
            m1_id = em.add(DVE, (lambda o=m1, s=b1: nc.vector.tensor_scalar(
                o, s, 4, None, SHR)), [dma_id], war_deps=scr.get(("m1", tr), []))
            t1_id = em.add(DVE, (lambda o=t1, s=b2: nc.vector.tensor_scalar(
                o, s, 16.0, None, MUL)), [dma_id], war_deps=scr.get(("t1", tr), []))
            t0_id = em.add(POOL, (lambda o=t0, s=m0: nc.gpsimd.tensor_scalar(
                o, s, 256.0, None, MUL)), [m0_id], war_deps=scr.get(("t0", tr), []))
            ve = tin_s[:, 2 * tr : K * 7 : 14]
            vo = tin_s[:, 2 * tr + 1 : K * 7 : 14]
            ve_id = em.add(POOL, (lambda o=ve, a=t0, c=b0: nc.gpsimd.tensor_tensor(
                o, a, c, ADD)), [t0_id, dma_id], war_deps=tin_war)
            vo_id = em.add(POOL, (lambda o=vo, a=t1, c=m1: nc.gpsimd.tensor_tensor(
                o, a, c, ADD)), [t1_id, m1_id, dma_id], war_deps=tin_war)
            writers[2 * tr] = ve_id
            writers[2 * tr + 1] = vo_id
            b["tq_readers"] += [m0_id, m1_id, t1_id, ve_id, vo_id]
            scr[("m0", tr)] = [t0_id]
            scr[("m1", tr)] = [vo_id]
            scr[("t0", tr)] = [ve_id]
            scr[("t1", tr)] = [vo_id]
        views = {}
        prod = {}
        for j in range(7):
            views[f"th{j}"] = tin_s[:, j : K * 7 : 7]
            prod[f"th{j}"] = [writers[j], writers[j + 7]]
        pv_s = b["pv"]
        outs = {
            "@p0": pv_s[:, 0 : K * 6 : 6],
            "@p1": pv_s[:, 1 : K * 6 : 6],
            "@p2": pv_s[:, 2 : K * 6 : 6],
            "@v0": pv_s[:, 3 : K * 6 : 6],
            "@v1": pv_s[:, 4 : K * 6 : 6],
            "@v2": pv_s[:, 5 : K * 6 : 6],
        }
        vts[v] = dict(b=b, t=t, views=views, prod=prod, outs=outs, owned={},
                      final_ids=[], store_war=list(b["store_ids"]))

    def finish_vt(v):
        tc = vts[v]
        b, t = tc["b"], tc["t"]
        sid = em.add(
            SP,
            (lambda s=b["pv"], t=t: nc.sync.dma_start(out=pv_t[t], in_=s)),
            list(tc["final_ids"]),
        )
        b["store_ids"] = [sid]

    def emit_op(i, v):
        tc = vts[v]
        views, prod, outs, owned = tc["views"], tc["prod"], tc["outs"], tc["owned"]
        op = ops[i]
        kind, out, ins = op[0], op[1], op[2]
        if kind in ("sin", "cvt"):
            engine = ACT
        elif kind in ("ata", "ts"):
            engine = DVE
        else:
            engine = POOL if out in GPSIMD_OPS else DVE

        deps = []
        for nm in ins:
            p = prod[nm]
            deps.extend(p if isinstance(p, list) else (p,))
        if out.startswith("@"):
            o = outs[out]
            war = list(tc["store_war"])  # can't overwrite staging mid-store
        else:
            SLACK = 10
            REG_CAP = 48
            pick = None
            for fi, (ap_, rd_) in enumerate(free):
                if all(em.frontier(fe) - fidx >= SLACK for fe, fidx in rd_):
                    pick = fi
                    break
            if pick is None and free and nreg[0] >= REG_CAP:
                pick = 0
            if pick is not None:
                o, war = free.pop(pick)
            else:
                o, war = new_reg(), []
            owned[out] = (o, [])

        if kind == "sin":
            scale, bias = op[3], op[4]

            def fn(o=o, s=views[ins[0]], scale=scale, bias=bias):
                return nc.scalar.activation(
                    o, s, SIN, bias=float(bias), scale=float(scale)
                )
        elif kind == "cvt":
            scale, bias = op[3], op[4]

            def fn(o=o, s=views[ins[0]], scale=scale, bias=bias):
                return nc.scalar.activation(
                    o, s, COPY, bias=float(bias), scale=float(scale)
                )
        elif kind == "ts":
            s_mul, s_add = op[3], op[4]

            def fn(o=o, s=views[ins[0]], s_mul=s_mul, s_add=s_add):
                return nc.vector.tensor_scalar(
                    o, s, float(s_mul), float(s_add), MUL, ADD
                )
        elif kind == "tt":
            alu = op[3]

            def fn(o=o, a=views[ins[0]], b=views[ins[1]], alu=alu, e=engine):
                eng = nc.gpsimd if e == POOL else nc.vector
                return eng.tensor_tensor(o, a, b, alu)
        else:
            s0, s1 = op[3], op[4]

            def fn(o=o, a=views[ins[0]], b=views[ins[1]], s0=s0, s1=s1):
                return nc.vector._custom_dve(
                    AFFINE_THEN_ADD, out=o, in0=a, in1=b, s0=float(s0), s1=float(s1)
                )

        op_id = em.add(engine, fn, deps, war_deps=war)
        if out.startswith("@"):
            tc["final_ids"].append(op_id)
        else:
            views[out] = o
            prod[out] = op_id

        for nm in ins:
            if nm.startswith("th"):
                tc["b"]["tin_readers"].append(op_id)
            if nm in owned:
                owned[nm][1].append(op_id)
                if last_use[nm] == i:
                    free.append((owned[nm][0], owned[nm][1]))
                    del owned[nm]

    OFF = globals().get("_OFF_OVERRIDE", 44)
    n_ops = len(ops)
    pending = {}
    emitted_ops = 0
    pos = 0
    started = 0
    base_pos = {}
    while emitted_ops < vt_total * n_ops:
        if started < vt_total and len(pending) < NBUF and (
            started == 0 or pos >= base_pos[started - 1] + OFF
        ):
            start_vt(started)
            pending[started] = 0
            base_pos[started] = pos
            started += 1
        for v in sorted(pending):
            j = pos - base_pos[v]
            if 0 <= pending[v] <= min(j, n_ops - 1):
                emit_op(pending[v], v)
                pending[v] += 1
                emitted_ops += 1
                if pending[v] == n_ops:
                    finish_vt(v)
                    del pending[v]
        pos += 1

    with ExitStack() as stack:
        sems = {
            SP: stack.enter_context(nc.semaphore("sp_sem")),
            ACT: stack.enter_context(nc.semaphore("act_sem")),
            DVE: stack.enter_context(nc.semaphore("dve_sem")),
            POOL: stack.enter_context(nc.semaphore("pool_sem")),
        }
        block = stack.enter_context(nc.Block())
        em.finalize(block, sems)
    return nc


_CACHE = {}   # nchunk -> compiled executable
_MESH = None
_SH = None
_DEVS = None


def _get_compiled(nchunk):
    global _MESH, _SH, _DEVS
    if nchunk in _CACHE:
        return _CACHE[nchunk]
    import jax
    import jax.numpy as jnp
    from jax.sharding import Mesh, PartitionSpec, NamedSharding
    try:
        from jax.experimental.shard_map import shard_map
    except ImportError:
        from jax.experimental import shard_map as _sm
        shard_map = _sm.shard_map
    from concourse import bass2jax

    bass2jax.install_neuronx_cc_hook()
    rows = B // (NCORES * nchunk)
    nc = _build(rows)

    if _DEVS is None:
        _DEVS = jax.devices()[:NCORES]
        _MESH = Mesh(np.asarray(_DEVS), ("core",))
        _SH = NamedSharding(_MESH, PartitionSpec("core"))

    out_aval = jax.core.ShapedArray((rows, 6), jnp.uint8)

    pname = nc.partition_id_tensor.name if nc.partition_id_tensor else None

    def _body(q):
        # partition_id must be the LAST operand: the Bass object declares a
        # partition_id ExternalInput, and neuronx_cc_hook's parameter-order
        # check drops operand_ids[:-1] assuming it.
        args = (q, bass2jax.partition_id_tensor()) if pname else (q,)
        in_names = ("th_q", pname) if pname else ("th_q",)
        (res,) = bass2jax._bass_exec_p.bind(
            *args,
            out_avals=(out_aval,),
            in_names=in_names,
            out_names=("pv",),
            lowering_input_output_aliases=(),
            sim_require_finite=True,
            sim_require_nnan=True,
            nc=nc,
        )
        return res

    fn = shard_map(
        _body,
        mesh=_MESH,
        in_specs=PartitionSpec("core"),
        out_specs=PartitionSpec("core"),
        check_rep=False,
    )

    def compile_fn():
        return (
            jax.jit(fn, in_shardings=_SH, out_shardings=_SH)
            .lower(jax.ShapeDtypeStruct((rows // 2 * NCORES, 21), jnp.uint8))
            .compile()
        )

    _CACHE[nchunk] = bass2jax.fast_dispatch_compile(compile_fn)
    return _CACHE[nchunk]


def _pack12(blk):
    """(rows, 7) f32 degrees -> (rows//2, 21) u8: 12-bit values, LE pairs."""
    v = np.rint((blk + np.float32(90.0)) * S12C).astype(np.uint16).reshape(-1, 2)
    out = np.empty((v.shape[0], 3), np.uint8)
    out[:, 0] = v[:, 0]
    out[:, 1] = (v[:, 0] >> 8) | ((v[:, 1] & 0x0F) << 4)
    out[:, 2] = v[:, 1] >> 4
    return out.reshape(-1, 21)


def kernel(thetas, nchunk=NCHUNK, nthreads=NTHREADS, block_puts=BLOCK_PUTS):
    import jax

    compiled = _get_compiled(nchunk)
    th = np.asarray(thetas)
    assert th.shape == (B, 7), th.shape
    cr = B // nchunk
    rows = cr // NCORES

    points = np.empty((B, 3), np.float32)
    vectors = np.empty((B, 3), np.float32)

    with ThreadPoolExecutor(nthreads) as pool:
        def put(g, c):
            blk = th[g * rows : (g + 1) * rows]
            q = _pack12(blk)
            r = jax.device_put(q, _DEVS[c])
            if block_puts:
                r.block_until_ready()
            return r

        put_futs = {}
        for ci in range(nchunk):
            for c in range(NCORES):
                put_futs[(ci, c)] = pool.submit(put, ci * NCORES + c, c)

        def fetch(ci, shard):
            s = np.asarray(shard.data).astype(np.float32)
            base = ci * cr + shard.index[0].start
            points[base : base + s.shape[0]] = (s[:, :3] - OFF_DE) * (1.0 / SP_)
            vectors[base : base + s.shape[0]] = (s[:, 3:] - OFF_DE) * (1.0 / SV_)

        fetch_futs = []
        for ci in range(nchunk):
            parts = [put_futs[(ci, c)].result() for c in range(NCORES)]
            ga = jax.make_array_from_single_device_arrays((cr // 2, 21), _SH, parts)
            out = compiled(ga)
            for shard in out.addressable_shards:
                fetch_futs.append(pool.submit(fetch, ci, shard))
        for f in fetch_futs:
            f.result()

    return points, vectors
